# revision 8
# baseline (speedup 1.0000x reference)
"""Trainium2 Bass kernel v2 for LongNet-style dilated attention.

Module config (hardcoded): x [4, 8192, 2048] f32, d_model=2048, 16 heads,
head_dim=128, segment=512, dilation=2.

Math per (batch, segment, head):
  g = x[b, seg, offset_h::2, h*128:(h+1)*128]          # [256, 128]
  A = softmax(g @ g.T / sqrt(128))                      # [256, 256]
  out[b, seg, offset_h::2, h*128:(h+1)*128] = A @ g     # rest stays 0

Sharding: 64 segments (4 batches x 16 segs) split 8-per-core across the
8 NeuronCores; segments are fully independent (no collectives).

Design (v2; v1 measured 169.6us on the grading harness):
  - All device I/O is bf16, host-packed into the exact layouts the
    engines consume (halves HBM traffic vs v1's fp32 loads/stores and
    makes every DMA fully contiguous / >=2KB-run):
      x  "pk"  [seg][u][blk][t][8*129]: token-major g per parity with a
         1.0 column baked after each head's 128 channels, so the A@g
         matmul rhs is a contiguous [128,129] slice whose last column
         accumulates the softmax denominator in the same matmul.
      xt "gT"  [seg][quad][c][head][t]: host-pretransposed channel-major
         copy; one 512KB DMA per (group, parity) lands all 8 heads'
         stationary S operands -- zero PE transposes, zero PSUM->SBUF
         copies (v1 spent ~35us/core of PE+DVE+ACT on those).
  - Per head-pair, S (2x[128,256]-col matmuls/head, bf16) and the out
    matmuls (4x n=129) share one PSUM tile [128,1024]: the out phase
    overwrites the S region after exp has read it (WAR tracked by the
    tile framework). 4 pair tiles = exactly 8 PSUM banks, which gives
    the pipeline a full round of slack on PSUM recycling.
  - One exp per pair ([128,1024], scale folded in) on ScalarE; its
    row-sums land in the den columns (col 128 of each 129-wide out
    block) via the baked ones.
  - Softmax normalize is ONE DVE tensor_tensor per pair: [128,(2hp),
    (2qc),128] strided PSUM read multiplied by a stride-0-broadcast
    [128,2,2] reciprocal tile, writing the bf16 store-stage directly.
  - Schedule (pair-indexed rounds): O(i-2), S(i), exp(i) same round
    (keeps ScalarE's queue fed one round ahead of the out matmuls),
    recip/norm(i-2), group loads 8-12 rounds ahead, one 512KB store
    per (group,u). ~18 warm-up matmuls run during the initial loads to
    climb the PE p-state ramp before real work arrives.

Measured (8x NC_v3 via axon, queued-dispatch differencing of reps=9 vs
reps=1 builds, min-statistic over noisy RPC walls): ~50us/core
steady-state per rep, consistent with the 55us/core PE floor (2056
MAC-optimal matmul cycles per head-pair at 2.4GHz). TimelineSim
cost-model single-shot estimate 80.9us (the cost model is conservative
vs real HW on ScalarE/DVE/DMA throughput for this mix). CoreSim + HW
rel err (absmax) ~3e-3 / rel L2 ~2.4e-3 vs the fp32 reference.
"""

import numpy as np
import ml_dtypes

import concourse.bacc as bacc
import concourse.bass as bass
import concourse.tile as tile
from concourse import mybir
from concourse.bass_utils import run_bass_kernel_spmd
from concourse.masks import make_identity

N_CORES = 8
B = 4
N_TOK = 8192
D = 2048
H = 16
HD = 128
SEG = 512
SDIL = 256
SCALE = 1.0 / float(np.sqrt(HD))

SEGS_TOTAL = (B * N_TOK) // SEG  # 64
NSEG = SEGS_TOTAL // N_CORES     # 8 per core

CW = HD + 1        # 129: per-head packed chunk (128 g cols + 1 ones col)
ROWC = 8 * CW      # 1032 packed row width

FP32 = mybir.dt.float32
BF16 = mybir.dt.bfloat16
EXP = mybir.ActivationFunctionType.Exp
NPBF16 = ml_dtypes.bfloat16

# quad modes per group, order Q = u*2 + qi (qi: which half of the 8 heads
# of parity u). "pe": transpose g on the PE; "hbm": DMA gT from the
# host-pretransposed copy.
TQ = ("hbm", "hbm", "hbm", "hbm")
HBM_QS = [q for q, m in enumerate(TQ) if m == "hbm"]
N_HQ = len(HBM_QS)
HQ_INDEX = {q: k for k, q in enumerate(HBM_QS)}
# how many PE-quad PSUM->SBUF copies go to the scalar engine instead of
# DVE (balance knob; count per 4 copies)
COPY_ACT_MOD = 0  # 0 = all on DVE
# DMA queue assignment (engine namespace names): pk loads, gT loads, stores
Q_PK = "sync"
Q_GT = "sync"
Q_ST = "sync"



def build_nc(n_segs=NSEG, reps=1):
    nc = bacc.Bacc(
        "TRN2", target_bir_lowering=False, debug=False, num_devices=N_CORES
    )
    x = nc.dram_tensor(
        "x", [n_segs * 2 * 2 * 128, ROWC], BF16, kind="ExternalInput"
    ).ap()
    xv = x.rearrange("(s u k t) w -> s u k t w", u=2, k=2, t=128)
    if N_HQ:
        xt = nc.dram_tensor(
            "xt", [n_segs * N_HQ * 128, 4 * 256], BF16, kind="ExternalInput"
        ).ap()
        xtv = xt.rearrange("(s q c) w -> s q c w", q=N_HQ, c=128)
    out = nc.dram_tensor(
        "out", [n_segs * 2 * 256, 1024], BF16, kind="ExternalOutput"
    ).ap()
    ov = out.rearrange("(s u i) w -> s u i w", u=2, i=256)

    n_pairs_per_seg_group = 8
    n_items = reps * n_segs * n_pairs_per_seg_group

    def info(i):
        g_abs, p = divmod(i, 8)
        u, hp = divmod(p, 4)
        return g_abs, u, hp

    def first_of_pe_quad(i):
        if not (0 <= i < n_items):
            return False
        _, u, hp = info(i)
        return hp % 2 == 0 and TQ[u * 2 + hp // 2] == "pe"

    with tile.TileContext(nc) as tc:
        with (
            tc.tile_pool(name="xb", bufs=3) as xb_pool,
            tc.tile_pool(name="gt", bufs=6) as gt_pool,
            tc.tile_pool(name="ee", bufs=4) as e_pool,
            tc.tile_pool(name="stage", bufs=3) as stage_pool,
            tc.tile_pool(name="rcp", bufs=4) as rcp_pool,
            tc.tile_pool(name="const", bufs=1) as const_pool,
            tc.tile_pool(name="gtps", bufs=2, space="PSUM") as gtps_pool,
            tc.tile_pool(name="pt", bufs=4, space="PSUM") as pt_pool,
        ):
            if any(m == "pe" for m in TQ):
                ident = const_pool.tile([128, 128], BF16)
                make_identity(nc, ident)
            warm = const_pool.tile([128, 128], BF16, tag="warm")
            nc.gpsimd.memset(warm, 0.0)
            wps = pt_pool.tile([128, 1024], FP32, tag="pt")
            for _ in range(18):
                nc.tensor.matmul(
                    wps[:, 0:128], warm, warm, start=True, stop=True
                )

            G = {}
            copy_ctr = [0]

            def emit_loads(g_abs, part):
                if not (0 <= g_abs < n_items // 8):
                    return
                gd = g_abs % n_segs
                u = part
                if part == 0:
                    xb = xb_pool.tile([128, 2, 2, ROWC], BF16, tag="xb")
                    G[("xb", g_abs)] = xb
                    st = stage_pool.tile([128, 2, 2, 8, HD], BF16, tag="st")
                    G[("st", g_abs)] = st
                xb = G[("xb", g_abs)]
                order = ("gt", "xb") if g_abs < 2 else ("xb", "gt")
                for what in order:
                    if what == "gt":
                        # one load covers both quads of this parity (8 heads)
                        gt = gt_pool.tile([128, 8, 256], BF16, tag="gt")
                        if g_abs == 0:
                            for qi in (0, 1):
                                src_ap = bass.AP(
                                    tensor=xtv.tensor,
                                    offset=xtv.offset
                                    + (gd * N_HQ + u * 2 + qi) * 128 * 1024,
                                    ap=[[1024, 128], [1, 1024]],
                                )
                                getattr(nc, Q_GT).dma_start(
                                    out=gt[:, qi * 4:qi * 4 + 4, :],
                                    in_=src_ap,
                                )
                        else:
                            src_ap = bass.AP(
                                tensor=xtv.tensor,
                                offset=xtv.offset
                                + (gd * N_HQ + u * 2) * 128 * 1024,
                                ap=[[1024, 128], [128 * 1024, 2], [1, 1024]],
                            )
                            getattr(nc, Q_GT).dma_start(out=gt, in_=src_ap)
                        G[("gtu", g_abs * 2 + u)] = gt
                    else:
                        xsrc = bass.AP(
                            tensor=xv.tensor,
                            offset=xv.offset + (gd * 2 + u) * 2 * 128 * ROWC,
                            ap=[[ROWC, 128], [128 * ROWC, 2], [1, ROWC]],
                        )
                        getattr(nc, Q_PK).dma_start(
                            out=xb[:, u, :, :], in_=xsrc
                        )

            def emit_T(i):
                # 8 PE transposes for the quad whose first pair is i
                g_abs, u, hp = info(i)
                qi = hp // 2
                xb = G[("xb", g_abs)]
                gtq = gtps_pool.tile([128, 4, 256], BF16, tag="gtq")
                for hj in range(4):
                    hi = qi * 4 + hj
                    cs = slice(hi * CW, hi * CW + HD)
                    for blk in (0, 1):
                        nc.tensor.transpose(
                            gtq[:, hj, blk * 128:(blk + 1) * 128],
                            xb[:, u, blk, cs],
                            ident,
                        )
                G[("gtq", i)] = gtq

            def emit_Tcopy(i):
                g_abs, u, hp = info(i)
                gtq = G.pop(("gtq", i))
                gt = gt_pool.tile([128, 4, 256], BF16, tag="gt")
                copy_ctr[0] += 1
                if COPY_ACT_MOD and copy_ctr[0] % COPY_ACT_MOD == 0:
                    nc.scalar.copy(gt, gtq)
                else:
                    nc.vector.tensor_copy(gt, gtq)
                G[("gt", g_abs * 4 + u * 2 + hp // 2)] = gt

            def emit_S(i):
                if not (0 <= i < n_items):
                    return
                g_abs, u, hp = info(i)
                gt = G.pop(("gtu", g_abs * 2 + u)) if hp == 3 else G[
                    ("gtu", g_abs * 2 + u)
                ]
                pt = pt_pool.tile([128, 1024], FP32, tag="pt")
                for jj in (0, 1):
                    gth = gt[:, hp * 2 + jj, :]
                    for qb in (0, 1):
                        nc.tensor.matmul(
                            pt[:, jj * 512 + qb * 256:jj * 512 + qb * 256 + 256],
                            gth[:, qb * 128:qb * 128 + 128],
                            gth,
                            start=True,
                            stop=True,
                        )
                G[("pt", i)] = pt

            def emit_exp(i):
                if not (0 <= i < n_items):
                    return
                pt = G[("pt", i)]
                e2 = e_pool.tile([128, 1024], BF16, tag="e")
                nc.scalar.activation(e2, pt, EXP, scale=SCALE)
                G[("e", i)] = e2

            def emit_O(i):
                if not (0 <= i < n_items):
                    return
                g_abs, u, hp = info(i)
                pt = G[("pt", i)]
                e2 = G.pop(("e", i))
                xb = G[("xb", g_abs)]
                for jj in (0, 1):
                    hi = hp * 2 + jj
                    for qc in (0, 1):
                        oap = pt[:, jj * 512 + qc * CW:jj * 512 + qc * CW + CW]
                        for tb in (0, 1):
                            nc.tensor.matmul(
                                oap,
                                e2[:, jj * 512 + tb * 256 + qc * 128:
                                   jj * 512 + tb * 256 + qc * 128 + 128],
                                xb[:, u, tb, hi * CW:hi * CW + CW],
                                start=(tb == 0),
                                stop=(tb == 1),
                            )

            def emit_recip(i):
                if not (0 <= i < n_items):
                    return
                pt = G[("pt", i)]
                rcp = rcp_pool.tile([128, 2, 2], FP32, tag="rcp")
                den = bass.AP(
                    tensor=pt.tensor,
                    offset=pt.offset + 128,
                    ap=[pt.ap[0], [512, 2], [CW, 2]],
                )
                nc.vector.reciprocal(rcp, den)
                G[("rcp", i)] = rcp

            def emit_norm(i):
                if not (0 <= i < n_items):
                    return
                g_abs, u, hp = info(i)
                pt = G.pop(("pt", i))
                rcp = G.pop(("rcp", i))
                st = G[("st", g_abs)]
                in0 = bass.AP(
                    tensor=pt.tensor,
                    offset=pt.offset,
                    ap=[pt.ap[0], [512, 2], [CW, 2], [1, 128]],
                )
                in1 = bass.AP(
                    tensor=rcp.tensor,
                    offset=rcp.offset,
                    ap=[rcp.ap[0], [2, 2], [1, 2], [0, 128]],
                )
                oap = bass.AP(
                    tensor=st.tensor,
                    offset=st.offset + u * 2048 + (hp * 2) * 128,
                    ap=[st.ap[0], [128, 2], [1024, 2], [1, 128]],
                )
                nc.vector.tensor_tensor(oap, in0, in1, mybir.AluOpType.mult)

            def emit_store(i):
                g_abs, u, hp = info(i)
                gd = g_abs % n_segs
                st = G.pop(("st", g_abs)) if u == 1 else G[("st", g_abs)]
                base = ov[gd, u]
                last = i == n_items - 1
                halves = ((0, 512), (512, 512)) if last else ((0, 1024),)
                for off, w in halves:
                    src = bass.AP(
                        tensor=st.tensor,
                        offset=st.offset + u * 2048 + off,
                        ap=[st.ap[0], [1024, 2], [1, w]],
                    )
                    dst = bass.AP(
                        tensor=base.tensor,
                        offset=base.offset + off,
                        ap=[[1024, 128], [128 * 1024, 2], [1, w]],
                    )
                    getattr(nc, Q_ST).dma_start(out=dst, in_=src)

            # prologue
            emit_loads(0, 0)
            emit_loads(0, 1)
            emit_loads(1, 0)
            if first_of_pe_quad(0):
                emit_T(0)
                emit_Tcopy(0)
            for i in range(n_items + 3):
                if i % 8 == 1:
                    emit_loads(i // 8 + 1, 1)
                if i % 8 == 4:
                    emit_loads(i // 8 + 2, 0)
                emit_O(i - 2)
                emit_S(i)
                emit_exp(i)
                if first_of_pe_quad(i + 2):
                    emit_T(i + 2)
                emit_recip(i - 2)
                emit_norm(i - 2)
                if first_of_pe_quad(i + 1):
                    emit_Tcopy(i + 1)
                if 0 <= i - 2 < n_items and (i - 2) % 4 == 3:
                    emit_store(i - 2)

    nc.compile()
    return nc


# ---------------- host-side packing ----------------

def pack_inputs(x: np.ndarray):
    """x [4, 8192, 2048] f32 -> (pk [64,2,2,128,ROWC] bf16, xt bf16)."""
    xr = np.ascontiguousarray(x).reshape(B, 16, 2, 128, 2, 8, 2, HD)
    # dims: b, s16, blk, t, u, hh, uu, c
    pk = np.empty((B, 16, 2, 2, 128, 8, CW), dtype=NPBF16)
    pk[..., HD] = np.asarray(1.0, NPBF16)
    xts = [None] * N_HQ
    for u in (0, 1):
        sel = xr[:, :, :, :, u, :, u, :]          # [b,s,blk,t,hh,c]
        selb = sel.astype(NPBF16)
        pk[:, :, u, :, :, :, :HD] = selb
        for qi in (0, 1):
            Q = u * 2 + qi
            if TQ[Q] != "hbm":
                continue
            gq = selb[:, :, :, :, 4 * qi:4 * qi + 4, :]   # [b,s,blk,t,hT,c]
            # -> [b, s, c, hT, blk, t]
            xts[HQ_INDEX[Q]] = np.ascontiguousarray(
                gq.transpose(0, 1, 5, 4, 2, 3)
            )
    pk = pk.reshape(SEGS_TOTAL, 2, 2, 128, ROWC)
    if N_HQ:
        xt = np.stack(xts, axis=2)   # [b, s, k, c, hT, blk, t]
        xt = xt.reshape(SEGS_TOTAL, N_HQ * 128, 1024)
    else:
        xt = None
    return pk, xt


def make_in_maps(x: np.ndarray):
    pk, xt = pack_inputs(x)
    in_maps = []
    for c in range(N_CORES):
        m = {
            "x": np.ascontiguousarray(
                pk[c * NSEG:(c + 1) * NSEG]
            ).reshape(NSEG * 2 * 2 * 128, ROWC)
        }
        if xt is not None:
            m["xt"] = np.ascontiguousarray(
                xt[c * NSEG:(c + 1) * NSEG]
            ).reshape(NSEG * N_HQ * 128, 1024)
        in_maps.append(m)
    return in_maps


def gather_out(results) -> np.ndarray:
    outs = [
        np.asarray(results[c]["out"]).reshape(NSEG, 2, 256, 1024)
        for c in range(N_CORES)
    ]
    opk = np.concatenate(outs, axis=0)  # [64, 2, 256, ROWC]
    full = np.zeros((B, N_TOK, D), np.float32)
    fullr = full.reshape(B, 16, 256, 2, 8, 2, HD)  # b s tdil u hh uu c
    for u in (0, 1):
        fullr[:, :, :, u, :, u, :] = (
            opk[:, u]
            .reshape(B, 16, 256, 8, HD)
            .astype(np.float32)
        )
    return full


_NC_CACHE = {}


def _get_nc():
    key = "full"
    if key not in _NC_CACHE:
        _NC_CACHE[key] = build_nc()
    return _NC_CACHE[key]


def kernel(x: np.ndarray) -> np.ndarray:
    assert x.shape == (B, N_TOK, D) and x.dtype == np.float32
    nc = _get_nc()
    in_maps = make_in_maps(x)
    last_err = None
    for _attempt in range(3):
        try:
            res = run_bass_kernel_spmd(nc, in_maps, list(range(N_CORES)))
            return gather_out(res.results)
        except Exception as e:  # transient NRT/device hiccup: retry
            last_err = e
    raise last_err
